# revision 2
# baseline (speedup 1.0000x reference)
"""Trainium2 Bass kernel for nn_BoundaryConsistencyLoss (v3).

loss = mean-over-valid-windows of mean-over-batch (pvar - tvar)^2 where
pvar/tvar are masked variances of sigmoid-probs / targets over sliding
windows of 5 along L.

Data-parallel over batch (512 = 8 cores x 64 rows). Per core the host pads
each input flat array by one window halo so that SBUF partition p = 2*b + h
(h = L-half) can be loaded with single 2-level-AP DMAs (128 x 16KB+
descriptor rows -> ~320 GB/s on the sync queue; 3-level APs serialize to
~1 DMA engine). Partition p's tail halo is row-contiguous in DRAM: real
continuation data for h=0 partitions, next row's head / zero pad for h=1
partitions, whose last 4 window slots are invalid and discarded on host.

Math per window (sums over 5): z=(2t+1)m, M=sum(m), Gm=sum(p^2 m - tm),
Hw=sum(pm - tm), Kw=sum(pm + tm), r=1/max(M,1):
  pvar - tvar = r*(Gm - r*Hw*Kw);  d2 = ((Gm - (Hw*Kw)*r)*r)^2
Validity (ref: sum_b M > 0) <=> sum_b Kw > 0 since p > 0 strictly.

Engine split per 2048-window chunk (HW-measured costs):
  sync-HWDGE: predictions pieces + output stores
  gpsimd SWDGE: targets/mask pieces with i32->bf16 cast (parallel queue)
  DVE: z2 (tensor_scalar), z (TT bf16), 4 fused custom scans (f32),
       RWIN custom (r = approx 1/max(cm5-cm0,1)), Gm diff, V/Q/T/S TT bf16
  Pool: dsg = x1-x0, Hw/Kw window diffs (plain tensor_tensor)
  Act: sigmoid, d2 = square, psum evacuations
  PE: ones-matmul batch reductions (bf16)
"""

import sys

if "/opt/trn_rl_repo" not in sys.path:
    sys.path.insert(0, "/opt/trn_rl_repo")

import numpy as np

import concourse.bass as bass
import concourse.tile as tile
from concourse import bacc, dve_ops, mybir
from concourse.bass_interp import get_hw_module
from concourse.bass_utils import run_bass_kernel_spmd
from concourse.dve_spec import (
    AluOp,
    Bin,
    C0,
    C1,
    C2,
    One,
    Spec,
    Src0,
    Src1,
    _has_src1,
    lower,
    maxx,
    minn,
    relu,
    scan,
    sq,
)
from concourse.dve_uop import DveOpSpec

F32 = mybir.dt.float32
BF16 = mybir.dt.bfloat16
I32 = mybir.dt.int32
AF = mybir.ActivationFunctionType
OP = mybir.AluOpType

NCORES = 8
B, L, C = 512, 16384, 2
BL = B // NCORES          # 64 batch rows per core
LH = L // 2               # 8192 per-half length
W = 5
NW = L - W + 1            # 16380 windows
P = 128

# variable chunk sizes: small edge chunks shrink pipeline fill/drain
CHUNKS = [(0, 512), (512, 512), (1024, 1024), (2048, 2048), (4096, 2048),
          (6144, 1024), (7168, 1024)]
CKMAX = 2048

# padded flat plane lengths (one tail halo so partition reads stay in-bounds)
TLEN = BL * L + (W - 1)

# piece boundaries (cols per partition)
TP = [0, 516, 1028, 2052, 4100, 6148, LH + 4]


def _f32(a):
    return np.asarray(a, np.float32)


def _register_op(name, spec, subdim=False):
    for op in dve_ops.OPS:
        if op.name == name:
            return op
    opcode = dve_ops._CUSTOM_DVE_ROW_BASE + len(dve_ops.OPS)
    shas = {}
    for ver in ("v3", "v4"):
        s = DveOpSpec(
            name=name, opcode=opcode, uops=lower(spec, ver=ver), rd1_en=_has_src1(spec)
        )
        shas[ver] = s.sha(ver)
    op = dve_ops.DveOp(name, spec, subdim=subdim, uops_sha=shas)
    dve_ops.OPS.append(op)
    dve_ops._SUB_OPCODE_FOR_NAME[name] = opcode
    dve_ops.CUSTOM_DVE_SPECS[name] = spec
    return op


# ---- fused cumsum ops (z = (2t+1)*m; m = min(z,1), tm = relu((z-1)*0.5)) ----
def _z_parts(z):
    z = _f32(z)
    return np.minimum(z, 1.0), np.maximum((z - 1.0) * np.float32(0.5), 0.0)


def _ref_mscan(in0, in1, s0, s1, imm2):
    return np.cumsum(np.minimum(_f32(in0), 1.0), axis=-1, dtype=np.float32)


def _ref_gscan(in0, in1, s0, s1, imm2):
    m, tm = _z_parts(in1)
    return np.cumsum(_f32(in0) * _f32(in0) * m - tm, axis=-1, dtype=np.float32)


def _ref_hscan(in0, in1, s0, s1, imm2):
    m, tm = _z_parts(in1)
    return np.cumsum(_f32(in0) * m - tm, axis=-1, dtype=np.float32)


def _ref_kscan(in0, in1, s0, s1, imm2):
    m, tm = _z_parts(in1)
    return np.cumsum(_f32(in0) * m + tm, axis=-1, dtype=np.float32)


_m_of_z = minn(Src1, One)
_tm_of_z = relu((Src1 - One) * C0)  # s0 = 0.5

MSCAN = _register_op(
    "BCL_MSCAN", Spec(body=scan(AluOp.ADD, minn(Src0, One)), reference=_ref_mscan)
)
GSCAN = _register_op(
    "BCL_GSCAN",
    Spec(body=scan(AluOp.ADD, sq(Src0) * _m_of_z - _tm_of_z), reference=_ref_gscan),
)
HSCAN = _register_op(
    "BCL_HSCAN",
    Spec(body=scan(AluOp.ADD, Src0 * _m_of_z - _tm_of_z), reference=_ref_hscan),
)
KSCAN = _register_op(
    "BCL_KSCAN",
    Spec(body=scan(AluOp.ADD, Src0 * _m_of_z + _tm_of_z), reference=_ref_kscan),
)

# ---- r = 1/max(Src0-Src1, 1): NOT-seed + one Newton pass, constants
# minimax-fitted to the exact domain d in {1..5} (max rel err 0.13%).
RW_CONSTS = {"s0": -0.2231725650808697, "s1": 1.8969666701661758,
             "imm2": 1.1139888028328857}


def _ref_rwin(in0, in1, c0, c1, c2):
    d = np.maximum(_f32(in0) - _f32(in1), 1.0)
    not_d = (~d.view(np.int32)).view(np.float32)
    y0 = not_d * c0
    return y0 * (c1 - d * y0) * c2


_d = maxx(Src0 - Src1, One)
_y0 = Bin(AluOp.BITWISE_NOT, _d, _d) * C0
RWIN = _register_op(
    "BCL_RWIN", Spec(body=(_y0 * (C1 - _d * _y0)) * C2, reference=_ref_rwin)
)


def _split_sync_waits(nc, max_waits=1):
    """walrus TPB_CTRL codegen rejects >1 explicit sem wait on Drain-class
    instructions; move excess waits onto preceding same-engine no-ops."""
    for fn in nc.m.functions:
        for bb in fn.blocks:
            new_insts = []
            for ins in bb.instructions:
                si = getattr(ins, "sync_info", None)
                waits = list(si.on_wait) if si is not None else []
                if len(waits) > max_waits:
                    extra, keep = waits[:-max_waits], waits[-max_waits:]
                    for j in range(0, len(extra), max_waits):
                        new_insts.append(
                            mybir.InstNoOp(
                                name=f"{ins.name}_wsplit{j}",
                                engine=ins.engine,
                                ins=[],
                                outs=[],
                                sync_info=mybir.SyncInfo(
                                    on_wait=extra[j : j + max_waits], on_update=[]
                                ),
                            )
                        )
                    si.on_wait.clear()
                    si.on_wait.extend(keep)
                new_insts.append(ins)
            bb.instructions = new_insts


def _build_program():
    nc = bacc.Bacc(
        "TRN2",
        target_bir_lowering=False,
        debug=False,
        enable_asserts=False,
        num_devices=NCORES,
    )
    x0d = nc.dram_tensor("x0", [1, TLEN], F32, kind="ExternalInput")
    x1d = nc.dram_tensor("x1", [1, TLEN], F32, kind="ExternalInput")
    tg = nc.dram_tensor("targets", [1, TLEN], I32, kind="ExternalInput")
    mk = nc.dram_tensor("mask", [1, TLEN], I32, kind="ExternalInput")
    ones_in = nc.dram_tensor("ones_const", [P, 2], BF16, kind="ExternalInput")
    ssd_o = nc.dram_tensor("ssd", [2, LH], BF16, kind="ExternalOutput")
    mst_o = nc.dram_tensor("mst", [2, LH], BF16, kind="ExternalOutput")

    with tile.TileContext(nc) as tc:
        with (
            tc.tile_pool(name="const", bufs=1) as const,
            tc.tile_pool(name="wsp", bufs=1) as wsp,      # f32 scan tiles
            tc.tile_pool(name="mid", bufs=2) as mid,      # bf16 integrands
            tc.tile_pool(name="cmb", bufs=2) as cmb,      # bf16 combine tiles
            tc.tile_pool(name="ev", bufs=2) as ev,
            tc.tile_pool(name="ps", bufs=2, space="PSUM") as ps,
        ):
            x0t = const.tile([P, LH + 4], BF16)
            x1t = const.tile([P, LH + 4], BF16)
            tb = const.tile([P, LH + 4], BF16)
            mb = const.tile([P, LH + 4], BF16)
            ones = const.tile([P, 2], BF16)

            nc.sync.dma_start(out=ones[:, :], in_=ones_in[:, :])

            def i_ap(t, lo, hi):
                return bass.AP(tensor=t[:, :].tensor, offset=lo,
                               ap=[[LH, P], [1, hi - lo]])

            for i in range(len(TP) - 1):
                lo, hi = TP[i], TP[i + 1]
                nc.gpsimd.dma_start(out=tb[:, lo:hi], in_=i_ap(tg, lo, hi))
                nc.gpsimd.dma_start(out=mb[:, lo:hi], in_=i_ap(mk, lo, hi))
                nc.gpsimd.dma_start(out=x0t[:, lo:hi], in_=i_ap(x0d, lo, hi))
                nc.gpsimd.dma_start(out=x1t[:, lo:hi], in_=i_ap(x1d, lo, hi))

            for c0, CK in CHUNKS:
                CKH = CK + (W - 1)

                dsg = mid.tile([P, CKMAX + 4], BF16, tag="dsg")
                nc.vector.tensor_sub(
                    dsg[:, :CKH], x1t[:, c0 : c0 + CKH], x0t[:, c0 : c0 + CKH]
                )
                p_t = mid.tile([P, CKMAX + 4], BF16, tag="p")
                nc.scalar.activation(p_t[:, :CKH], dsg[:, :CKH], AF.Sigmoid)

                z2 = mid.tile([P, CKMAX + 4], BF16, tag="z2")
                nc.scalar.activation(z2[:, :CKH], tb[:, c0 : c0 + CKH],
                                     AF.Identity, bias=1.0, scale=2.0)
                z = mid.tile([P, CKMAX + 4], BF16, tag="z")
                nc.vector.tensor_mul(z[:, :CKH], z2[:, :CKH],
                                     mb[:, c0 : c0 + CKH])

                cts = {}
                for nm, op_, args in (
                    ("m", MSCAN, dict(in0=z[:, :CKH])),
                    ("g", GSCAN, dict(in0=p_t[:, :CKH], in1=z[:, :CKH], s0=0.5)),
                    ("h", HSCAN, dict(in0=p_t[:, :CKH], in1=z[:, :CKH], s0=0.5)),
                    ("k", KSCAN, dict(in0=p_t[:, :CKH], in1=z[:, :CKH], s0=0.5)),
                ):
                    ct = wsp.tile([P, CKMAX + 5], F32, tag=f"c_{nm}")
                    nc.vector.memset(ct[:, 0:1], 0.0)
                    nc.vector._custom_dve(op_, out=ct[:, 1 : CKH + 1], **args)
                    cts[nm] = ct

                r = cmb.tile([P, CKMAX], BF16, tag="r")
                nc.vector._custom_dve(
                    RWIN, out=r[:, :CK], in0=cts["m"][:, W : CK + W],
                    in1=cts["m"][:, 0:CK],
                    s0=RW_CONSTS["s0"], s1=RW_CONSTS["s1"], imm2=RW_CONSTS["imm2"],
                )
                Gm = cmb.tile([P, CKMAX], BF16, tag="Gm")
                nc.vector.tensor_sub(Gm[:, :CK], cts["g"][:, W : CK + W],
                                     cts["g"][:, 0:CK])
                Hw = cmb.tile([P, CKMAX], BF16, tag="Hw")
                nc.vector.tensor_sub(Hw[:, :CK], cts["h"][:, W : CK + W],
                                     cts["h"][:, 0:CK])
                Kw = cmb.tile([P, CKMAX], BF16, tag="Kw")
                nc.vector.tensor_sub(Kw[:, :CK], cts["k"][:, W : CK + W],
                                     cts["k"][:, 0:CK])

                W1 = cmb.tile([P, CKMAX], BF16, tag="W1")
                nc.vector.tensor_mul(W1[:, :CK], Hw[:, :CK], Kw[:, :CK])
                nc.vector.tensor_mul(W1[:, :CK], W1[:, :CK], r[:, :CK])
                nc.vector.tensor_sub(W1[:, :CK], Gm[:, :CK], W1[:, :CK])
                nc.vector.tensor_mul(W1[:, :CK], W1[:, :CK], r[:, :CK])
                d2 = cmb.tile([P, CKMAX], BF16, tag="d2")
                nc.scalar.square(d2[:, :CK], W1[:, :CK])

                NQ = max(CK // 1024, 1)
                QW = CK // NQ
                for q in range(NQ):
                    mst_ps = ps.tile([2, 1024], F32, tag="mstp")
                    ssd_ps = ps.tile([2, 1024], F32, tag="ssdp")
                    for h in range(QW // 512):
                        sl = slice(q * QW + h * 512, q * QW + (h + 1) * 512)
                        psl = slice(h * 512, (h + 1) * 512)
                        nc.tensor.matmul(mst_ps[:, psl], ones[:, :], Kw[:, sl],
                                         start=True, stop=True)
                        nc.tensor.matmul(ssd_ps[:, psl], ones[:, :], d2[:, sl],
                                         start=True, stop=True)
                    ssd_ev = ev.tile([2, 1024], BF16, tag="ssd_ev")
                    nc.scalar.copy(ssd_ev[:, :QW], ssd_ps[:, :QW])
                    mst_ev = ev.tile([2, 1024], BF16, tag="mst_ev")
                    nc.scalar.copy(mst_ev[:, :QW], mst_ps[:, :QW])
                    nc.sync.dma_start(
                        out=ssd_o[:, c0 + q * QW : c0 + (q + 1) * QW],
                        in_=ssd_ev[:, :QW])
                    nc.sync.dma_start(
                        out=mst_o[:, c0 + q * QW : c0 + (q + 1) * QW],
                        in_=mst_ev[:, :QW])

    nc.compile()
    nc.m = get_hw_module(nc.m)
    _split_sync_waits(nc)
    return nc


_NC_CACHE = {}


def _get_nc():
    if "nc" not in _NC_CACHE:
        _NC_CACHE["nc"] = _build_program()
    return _NC_CACHE["nc"]


def _ones_pattern():
    import ml_dtypes

    o = np.zeros((P, 2), np.float32)
    o[0::2, 0] = 1.0   # even partitions: h=0 rows
    o[1::2, 1] = 1.0   # odd partitions: h=1 rows
    return o.astype(ml_dtypes.bfloat16)


def run_on_device(predictions, targets, mask, **spmd_kwargs):
    nc = _get_nc()
    predictions = np.asarray(predictions, np.float32)
    targets = np.asarray(targets, np.int32)
    mask = np.asarray(mask, np.int32)
    ones = _ones_pattern()
    in_maps = []
    zp = np.zeros(W - 1, np.float32)
    zi = np.zeros(W - 1, np.int32)
    for i in range(NCORES):
        sl = slice(i * BL, (i + 1) * BL)
        in_maps.append(
            {
                "x0": np.concatenate(
                    [np.ascontiguousarray(predictions[sl, :, 0]).ravel(),
                     zp])[None, :],
                "x1": np.concatenate(
                    [np.ascontiguousarray(predictions[sl, :, 1]).ravel(),
                     zp])[None, :],
                "targets": np.concatenate([targets[sl].ravel(), zi])[None, :],
                "mask": np.concatenate([mask[sl].ravel(), zi])[None, :],
                "ones_const": ones,
            }
        )
    return run_bass_kernel_spmd(nc, in_maps, core_ids=list(range(NCORES)),
                                **spmd_kwargs)


def combine_host(results):
    ssd_tot = np.zeros(NW, np.float64)
    mst_tot = np.zeros(NW, np.float64)
    for out in results:
        ssd = np.asarray(out["ssd"], np.float32)
        mst = np.asarray(out["mst"], np.float32)
        ssd_tot += np.concatenate([ssd[0], ssd[1][: NW - LH]])
        mst_tot += np.concatenate([mst[0], mst[1][: NW - LH]])
    mse = ssd_tot / B
    valid = (mst_tot > 0).astype(np.float64)
    cnt = max(valid.sum(), 1.0)
    loss = (mse * valid).sum() / cnt
    return np.asarray(loss, dtype=np.float32)


def kernel(predictions, targets, mask):
    res = run_on_device(predictions, targets, mask)
    return combine_host(res.results)


if __name__ == "__main__":
    rng = np.random.default_rng(0)
    p = rng.standard_normal((B, L, C), dtype=np.float32)
    t = rng.integers(0, 2, (B, L)).astype(np.int32)
    m = rng.integers(0, 2, (B, L)).astype(np.int32)
    print(kernel(p, t, m))


# revision 3
# speedup vs baseline: 1.1525x; 1.1525x over previous
"""Trainium2 Bass kernel for nn_BoundaryConsistencyLoss (v3).

loss = mean-over-valid-windows of mean-over-batch (pvar - tvar)^2 where
pvar/tvar are masked variances of sigmoid-probs / targets over sliding
windows of 5 along L.

Data-parallel over batch (512 = 8 cores x 64 rows). Per core the host pads
each input flat array by one window halo so that SBUF partition p = 2*b + h
(h = L-half) can be loaded with single 2-level-AP DMAs (128 x 16KB+
descriptor rows -> ~320 GB/s on the sync queue; 3-level APs serialize to
~1 DMA engine). Partition p's tail halo is row-contiguous in DRAM: real
continuation data for h=0 partitions, next row's head / zero pad for h=1
partitions, whose last 4 window slots are invalid and discarded on host.

Math per window (sums over 5): z=(2t+1)m, M=sum(m), Gm=sum(p^2 m - tm),
Hw=sum(pm - tm), Kw=sum(pm + tm), r=1/max(M,1):
  pvar - tvar = r*(Gm - r*Hw*Kw);  d2 = ((Gm - (Hw*Kw)*r)*r)^2
Validity (ref: sum_b M > 0) <=> sum_b Kw > 0 since p > 0 strictly.

Engine split per chunk (HW-measured costs; DVE+Act coexist cleanly but
DVE+Pool ops mutually slow ~2x on SBUF ports, so Pool only runs SWDGE):
  gpsimd SWDGE queue: all four input planes cast-loaded f32/i32 -> bf16
      (halves SBUF write pressure; host deinterleaves prediction channels
      so dsg is a packed-bf16 2x tensor_tensor)
  sync-HWDGE queue: ones constant + output stores
  DVE: dsg, z = (2t+1)m, 4 fused custom cumsum scans (f32), RWIN custom
      (r ~ 1/max(Mw,1), Newton constants fitted to M in {1..5}), window
      diffs (f32->bf16), V/Q/T/S combine chain in one bf16 tile
  Act: z2 = 2t+1, sigmoid, d2 = square, psum evacuations
  PE: ones-matmul batch reductions over partitions (bf16)
Variable chunk sizes (512/1024/2048) shrink pipeline fill and drain.
"""

import sys

if "/opt/trn_rl_repo" not in sys.path:
    sys.path.insert(0, "/opt/trn_rl_repo")

import numpy as np

import concourse.bass as bass
import concourse.tile as tile
from concourse import bacc, dve_ops, mybir
from concourse.bass_interp import get_hw_module
from concourse.bass_utils import run_bass_kernel_spmd
from concourse.dve_spec import (
    AluOp,
    Bin,
    C0,
    C1,
    C2,
    One,
    Spec,
    Src0,
    Src1,
    _has_src1,
    lower,
    maxx,
    minn,
    relu,
    scan,
    sq,
)
from concourse.dve_uop import DveOpSpec

F32 = mybir.dt.float32
BF16 = mybir.dt.bfloat16
I32 = mybir.dt.int32
AF = mybir.ActivationFunctionType
OP = mybir.AluOpType

NCORES = 8
B, L, C = 512, 16384, 2
BL = B // NCORES          # 64 batch rows per core
LH = L // 2               # 8192 per-half length
W = 5
NW = L - W + 1            # 16380 windows
P = 128

# variable chunk sizes: small edge chunks shrink pipeline fill/drain
CHUNKS = [(0, 512), (512, 512), (1024, 1024), (2048, 2048), (4096, 2048),
          (6144, 1024), (7168, 1024)]
CKMAX = 2048

# padded flat plane lengths (one tail halo so partition reads stay in-bounds)
TLEN = BL * L + (W - 1)

# piece boundaries (cols per partition)
TP = [0, 516, 1028, 2052, 4100, 6148, LH + 4]


def _f32(a):
    return np.asarray(a, np.float32)


def _register_op(name, spec, subdim=False):
    for op in dve_ops.OPS:
        if op.name == name:
            return op
    opcode = dve_ops._CUSTOM_DVE_ROW_BASE + len(dve_ops.OPS)
    shas = {}
    for ver in ("v3", "v4"):
        s = DveOpSpec(
            name=name, opcode=opcode, uops=lower(spec, ver=ver), rd1_en=_has_src1(spec)
        )
        shas[ver] = s.sha(ver)
    op = dve_ops.DveOp(name, spec, subdim=subdim, uops_sha=shas)
    dve_ops.OPS.append(op)
    dve_ops._SUB_OPCODE_FOR_NAME[name] = opcode
    dve_ops.CUSTOM_DVE_SPECS[name] = spec
    return op


# ---- fused cumsum ops (z = (2t+1)*m; m = min(z,1), tm = relu((z-1)*0.5)) ----
def _z_parts(z):
    z = _f32(z)
    return np.minimum(z, 1.0), np.maximum((z - 1.0) * np.float32(0.5), 0.0)


def _ref_mscan(in0, in1, s0, s1, imm2):
    return np.cumsum(np.minimum(_f32(in0), 1.0), axis=-1, dtype=np.float32)


def _ref_gscan(in0, in1, s0, s1, imm2):
    m, tm = _z_parts(in1)
    return np.cumsum(_f32(in0) * _f32(in0) * m - tm, axis=-1, dtype=np.float32)


def _ref_hscan(in0, in1, s0, s1, imm2):
    m, tm = _z_parts(in1)
    return np.cumsum(_f32(in0) * m - tm, axis=-1, dtype=np.float32)


def _ref_kscan(in0, in1, s0, s1, imm2):
    m, tm = _z_parts(in1)
    return np.cumsum(_f32(in0) * m + tm, axis=-1, dtype=np.float32)


_m_of_z = minn(Src1, One)
_tm_of_z = relu((Src1 - One) * C0)  # s0 = 0.5

MSCAN = _register_op(
    "BCL_MSCAN", Spec(body=scan(AluOp.ADD, minn(Src0, One)), reference=_ref_mscan)
)
GSCAN = _register_op(
    "BCL_GSCAN",
    Spec(body=scan(AluOp.ADD, sq(Src0) * _m_of_z - _tm_of_z), reference=_ref_gscan),
)
HSCAN = _register_op(
    "BCL_HSCAN",
    Spec(body=scan(AluOp.ADD, Src0 * _m_of_z - _tm_of_z), reference=_ref_hscan),
)
KSCAN = _register_op(
    "BCL_KSCAN",
    Spec(body=scan(AluOp.ADD, Src0 * _m_of_z + _tm_of_z), reference=_ref_kscan),
)

# ---- r = 1/max(Src0-Src1, 1): NOT-seed + one Newton pass, constants
# minimax-fitted to the exact domain d in {1..5} (max rel err 0.13%).
RW_CONSTS = {"s0": -0.2231725650808697, "s1": 1.8969666701661758,
             "imm2": 1.1139888028328857}


def _ref_rwin(in0, in1, c0, c1, c2):
    d = np.maximum(_f32(in0) - _f32(in1), 1.0)
    not_d = (~d.view(np.int32)).view(np.float32)
    y0 = not_d * c0
    return y0 * (c1 - d * y0) * c2


_d = maxx(Src0 - Src1, One)
_y0 = Bin(AluOp.BITWISE_NOT, _d, _d) * C0
RWIN = _register_op(
    "BCL_RWIN", Spec(body=(_y0 * (C1 - _d * _y0)) * C2, reference=_ref_rwin)
)


def _split_sync_waits(nc, max_waits=1):
    """walrus TPB_CTRL codegen rejects >1 explicit sem wait on Drain-class
    instructions; move excess waits onto preceding same-engine no-ops."""
    for fn in nc.m.functions:
        for bb in fn.blocks:
            new_insts = []
            for ins in bb.instructions:
                si = getattr(ins, "sync_info", None)
                waits = list(si.on_wait) if si is not None else []
                if len(waits) > max_waits:
                    extra, keep = waits[:-max_waits], waits[-max_waits:]
                    for j in range(0, len(extra), max_waits):
                        new_insts.append(
                            mybir.InstNoOp(
                                name=f"{ins.name}_wsplit{j}",
                                engine=ins.engine,
                                ins=[],
                                outs=[],
                                sync_info=mybir.SyncInfo(
                                    on_wait=extra[j : j + max_waits], on_update=[]
                                ),
                            )
                        )
                    si.on_wait.clear()
                    si.on_wait.extend(keep)
                new_insts.append(ins)
            bb.instructions = new_insts


def _build_program():
    nc = bacc.Bacc(
        "TRN2",
        target_bir_lowering=False,
        debug=False,
        enable_asserts=False,
        num_devices=NCORES,
    )
    x0d = nc.dram_tensor("x0", [1, TLEN], F32, kind="ExternalInput")
    x1d = nc.dram_tensor("x1", [1, TLEN], F32, kind="ExternalInput")
    tg = nc.dram_tensor("targets", [1, TLEN], I32, kind="ExternalInput")
    mk = nc.dram_tensor("mask", [1, TLEN], I32, kind="ExternalInput")
    ones_in = nc.dram_tensor("ones_const", [P, 2], BF16, kind="ExternalInput")
    ssd_o = nc.dram_tensor("ssd", [2, LH], BF16, kind="ExternalOutput")
    mst_o = nc.dram_tensor("mst", [2, LH], BF16, kind="ExternalOutput")

    with tile.TileContext(nc) as tc:
        with (
            tc.tile_pool(name="const", bufs=1) as const,
            tc.tile_pool(name="wsp", bufs=1) as wsp,      # f32 scan tiles
            tc.tile_pool(name="mid", bufs=2) as mid,      # bf16 integrands
            tc.tile_pool(name="cmb", bufs=2) as cmb,      # bf16 combine tiles
            tc.tile_pool(name="ev", bufs=2) as ev,
            tc.tile_pool(name="ps", bufs=2, space="PSUM") as ps,
        ):
            x0t = const.tile([P, LH + 4], BF16)
            x1t = const.tile([P, LH + 4], BF16)
            tb = const.tile([P, LH + 4], BF16)
            mb = const.tile([P, LH + 4], BF16)
            ones = const.tile([P, 2], BF16)

            nc.sync.dma_start(out=ones[:, :], in_=ones_in[:, :])

            def i_ap(t, lo, hi):
                return bass.AP(tensor=t[:, :].tensor, offset=lo,
                               ap=[[LH, P], [1, hi - lo]])

            for i in range(len(TP) - 1):
                lo, hi = TP[i], TP[i + 1]
                nc.gpsimd.dma_start(out=tb[:, lo:hi], in_=i_ap(tg, lo, hi))
                nc.gpsimd.dma_start(out=mb[:, lo:hi], in_=i_ap(mk, lo, hi))
                nc.gpsimd.dma_start(out=x0t[:, lo:hi], in_=i_ap(x0d, lo, hi))
                nc.gpsimd.dma_start(out=x1t[:, lo:hi], in_=i_ap(x1d, lo, hi))

            for c0, CK in CHUNKS:
                CKH = CK + (W - 1)

                dsg = mid.tile([P, CKMAX + 4], BF16, tag="dsg")
                nc.vector.tensor_sub(
                    dsg[:, :CKH], x1t[:, c0 : c0 + CKH], x0t[:, c0 : c0 + CKH]
                )
                p_t = mid.tile([P, CKMAX + 4], BF16, tag="p")
                nc.scalar.activation(p_t[:, :CKH], dsg[:, :CKH], AF.Sigmoid)

                z2 = mid.tile([P, CKMAX + 4], BF16, tag="z2")
                nc.scalar.activation(z2[:, :CKH], tb[:, c0 : c0 + CKH],
                                     AF.Identity, bias=1.0, scale=2.0)
                z = mid.tile([P, CKMAX + 4], BF16, tag="z")
                nc.vector.tensor_mul(z[:, :CKH], z2[:, :CKH],
                                     mb[:, c0 : c0 + CKH])

                cts = {}
                for nm, op_, args in (
                    ("m", MSCAN, dict(in0=z[:, :CKH])),
                    ("g", GSCAN, dict(in0=p_t[:, :CKH], in1=z[:, :CKH], s0=0.5)),
                    ("h", HSCAN, dict(in0=p_t[:, :CKH], in1=z[:, :CKH], s0=0.5)),
                    ("k", KSCAN, dict(in0=p_t[:, :CKH], in1=z[:, :CKH], s0=0.5)),
                ):
                    ct = wsp.tile([P, CKMAX + 5], F32, tag=f"c_{nm}")
                    nc.vector.memset(ct[:, 0:1], 0.0)
                    nc.vector._custom_dve(op_, out=ct[:, 1 : CKH + 1], **args)
                    cts[nm] = ct

                r = cmb.tile([P, CKMAX], BF16, tag="r")
                nc.vector._custom_dve(
                    RWIN, out=r[:, :CK], in0=cts["m"][:, W : CK + W],
                    in1=cts["m"][:, 0:CK],
                    s0=RW_CONSTS["s0"], s1=RW_CONSTS["s1"], imm2=RW_CONSTS["imm2"],
                )
                Gm = cmb.tile([P, CKMAX], BF16, tag="Gm")
                nc.vector.tensor_sub(Gm[:, :CK], cts["g"][:, W : CK + W],
                                     cts["g"][:, 0:CK])
                Hw = cmb.tile([P, CKMAX], BF16, tag="Hw")
                nc.vector.tensor_sub(Hw[:, :CK], cts["h"][:, W : CK + W],
                                     cts["h"][:, 0:CK])
                Kw = cmb.tile([P, CKMAX], BF16, tag="Kw")
                nc.vector.tensor_sub(Kw[:, :CK], cts["k"][:, W : CK + W],
                                     cts["k"][:, 0:CK])

                W1 = cmb.tile([P, CKMAX], BF16, tag="W1")
                nc.vector.tensor_mul(W1[:, :CK], Hw[:, :CK], Kw[:, :CK])
                nc.vector.tensor_mul(W1[:, :CK], W1[:, :CK], r[:, :CK])
                nc.vector.tensor_sub(W1[:, :CK], Gm[:, :CK], W1[:, :CK])
                nc.vector.tensor_mul(W1[:, :CK], W1[:, :CK], r[:, :CK])
                d2 = cmb.tile([P, CKMAX], BF16, tag="d2")
                nc.scalar.square(d2[:, :CK], W1[:, :CK])

                NQ = max(CK // 1024, 1)
                QW = CK // NQ
                for q in range(NQ):
                    mst_ps = ps.tile([2, 1024], F32, tag="mstp")
                    ssd_ps = ps.tile([2, 1024], F32, tag="ssdp")
                    for h in range(QW // 512):
                        sl = slice(q * QW + h * 512, q * QW + (h + 1) * 512)
                        psl = slice(h * 512, (h + 1) * 512)
                        nc.tensor.matmul(mst_ps[:, psl], ones[:, :], Kw[:, sl],
                                         start=True, stop=True)
                        nc.tensor.matmul(ssd_ps[:, psl], ones[:, :], d2[:, sl],
                                         start=True, stop=True)
                    ssd_ev = ev.tile([2, 1024], BF16, tag="ssd_ev")
                    nc.scalar.copy(ssd_ev[:, :QW], ssd_ps[:, :QW])
                    mst_ev = ev.tile([2, 1024], BF16, tag="mst_ev")
                    nc.scalar.copy(mst_ev[:, :QW], mst_ps[:, :QW])
                    nc.sync.dma_start(
                        out=ssd_o[:, c0 + q * QW : c0 + (q + 1) * QW],
                        in_=ssd_ev[:, :QW])
                    nc.sync.dma_start(
                        out=mst_o[:, c0 + q * QW : c0 + (q + 1) * QW],
                        in_=mst_ev[:, :QW])

    nc.compile()
    nc.m = get_hw_module(nc.m)
    _split_sync_waits(nc)
    return nc


_NC_CACHE = {}


def _get_nc():
    if "nc" not in _NC_CACHE:
        _NC_CACHE["nc"] = _build_program()
    return _NC_CACHE["nc"]


def _ones_pattern():
    import ml_dtypes

    o = np.zeros((P, 2), np.float32)
    o[0::2, 0] = 1.0   # even partitions: h=0 rows
    o[1::2, 1] = 1.0   # odd partitions: h=1 rows
    return o.astype(ml_dtypes.bfloat16)


def run_on_device(predictions, targets, mask, **spmd_kwargs):
    nc = _get_nc()
    predictions = np.asarray(predictions, np.float32)
    targets = np.asarray(targets, np.int32)
    mask = np.asarray(mask, np.int32)
    ones = _ones_pattern()
    in_maps = []
    zp = np.zeros(W - 1, np.float32)
    zi = np.zeros(W - 1, np.int32)
    for i in range(NCORES):
        sl = slice(i * BL, (i + 1) * BL)
        in_maps.append(
            {
                "x0": np.concatenate(
                    [np.ascontiguousarray(predictions[sl, :, 0]).ravel(),
                     zp])[None, :],
                "x1": np.concatenate(
                    [np.ascontiguousarray(predictions[sl, :, 1]).ravel(),
                     zp])[None, :],
                "targets": np.concatenate([targets[sl].ravel(), zi])[None, :],
                "mask": np.concatenate([mask[sl].ravel(), zi])[None, :],
                "ones_const": ones,
            }
        )
    return run_bass_kernel_spmd(nc, in_maps, core_ids=list(range(NCORES)),
                                **spmd_kwargs)


def combine_host(results):
    ssd_tot = np.zeros(NW, np.float64)
    mst_tot = np.zeros(NW, np.float64)
    for out in results:
        ssd = np.asarray(out["ssd"], np.float32)
        mst = np.asarray(out["mst"], np.float32)
        ssd_tot += np.concatenate([ssd[0], ssd[1][: NW - LH]])
        mst_tot += np.concatenate([mst[0], mst[1][: NW - LH]])
    mse = ssd_tot / B
    valid = (mst_tot > 0).astype(np.float64)
    cnt = max(valid.sum(), 1.0)
    loss = (mse * valid).sum() / cnt
    return np.asarray(loss, dtype=np.float32)


def kernel(predictions, targets, mask):
    res = run_on_device(predictions, targets, mask)
    return combine_host(res.results)


if __name__ == "__main__":
    rng = np.random.default_rng(0)
    p = rng.standard_normal((B, L, C), dtype=np.float32)
    t = rng.integers(0, 2, (B, L)).astype(np.int32)
    m = rng.integers(0, 2, (B, L)).astype(np.int32)
    print(kernel(p, t, m))


# revision 4
# speedup vs baseline: 1.1800x; 1.0239x over previous
"""Trainium2 Bass kernel for nn_BoundaryConsistencyLoss (v3).

loss = mean-over-valid-windows of mean-over-batch (pvar - tvar)^2 where
pvar/tvar are masked variances of sigmoid-probs / targets over sliding
windows of 5 along L.

Data-parallel over batch (512 = 8 cores x 64 rows). Per core the host pads
each input flat array by one window halo so that SBUF partition p = 2*b + h
(h = L-half) can be loaded with single 2-level-AP DMAs (128 x 16KB+
descriptor rows -> ~320 GB/s on the sync queue; 3-level APs serialize to
~1 DMA engine). Partition p's tail halo is row-contiguous in DRAM: real
continuation data for h=0 partitions, next row's head / zero pad for h=1
partitions, whose last 4 window slots are invalid and discarded on host.

Math per window (sums over 5): z=(2t+1)m, M=sum(m), Gm=sum(p^2 m - tm),
Hw=sum(pm - tm), Kw=sum(pm + tm), r=1/max(M,1):
  pvar - tvar = r*(Gm - r*Hw*Kw);  d2 = ((Gm - (Hw*Kw)*r)*r)^2
Validity (ref: sum_b M > 0) <=> sum_b Kw > 0 since p > 0 strictly.

Engine split per chunk (HW-measured costs; DVE+Act coexist cleanly but
DVE+Pool ops mutually slow ~2x on SBUF ports, so Pool only runs SWDGE):
  gpsimd SWDGE queue: all four input planes cast-loaded f32/i32 -> bf16
      (halves SBUF write pressure; host deinterleaves prediction channels
      so dsg is a packed-bf16 2x tensor_tensor)
  sync-HWDGE queue: ones constant + output stores
  DVE: dsg, z = (2t+1)m, 4 fused custom cumsum scans (f32), RWIN custom
      (r ~ 1/max(Mw,1), Newton constants fitted to M in {1..5}), window
      diffs (f32->bf16), V/Q/T/S combine chain in one bf16 tile
  Act: z2 = 2t+1, sigmoid, d2 = square, psum evacuations
  PE: ones-matmul batch reductions over partitions (bf16)
Variable chunk sizes (512/1024/2048) shrink pipeline fill and drain.
"""

import sys

if "/opt/trn_rl_repo" not in sys.path:
    sys.path.insert(0, "/opt/trn_rl_repo")

import numpy as np

import concourse.bass as bass
import concourse.tile as tile
from concourse import bacc, dve_ops, mybir
from concourse.bass_interp import get_hw_module
from concourse.bass_utils import run_bass_kernel_spmd
from concourse.dve_spec import (
    AluOp,
    Bin,
    C0,
    C1,
    C2,
    One,
    Spec,
    Src0,
    Src1,
    _has_src1,
    lower,
    maxx,
    minn,
    relu,
    scan,
    sq,
)
from concourse.dve_uop import DveOpSpec

F32 = mybir.dt.float32
F16 = mybir.dt.float16
BF16 = mybir.dt.bfloat16
I32 = mybir.dt.int32
AF = mybir.ActivationFunctionType
OP = mybir.AluOpType

NCORES = 8
B, L, C = 512, 16384, 2
BL = B // NCORES          # 64 batch rows per core
LH = L // 2               # 8192 per-half length
W = 5
NW = L - W + 1            # 16380 windows
P = 128

# variable chunk sizes: small edge chunks shrink pipeline fill/drain
CHUNKS = [(0, 512), (512, 512)] + [(k, 1024) for k in range(1024, 7168, 1024)] \
    + [(7168, 512), (7680, 512)]
CKMAX = 1024

# padded flat plane lengths (one tail halo so partition reads stay in-bounds)
TLEN = BL * L + (W - 1)

# piece boundaries (cols per partition)
TP = [0, 516, 1028, 2052, 4100, 6148, LH + 4]


def _f32(a):
    return np.asarray(a, np.float32)


def _register_op(name, spec, subdim=False):
    for op in dve_ops.OPS:
        if op.name == name:
            return op
    opcode = dve_ops._CUSTOM_DVE_ROW_BASE + len(dve_ops.OPS)
    shas = {}
    for ver in ("v3", "v4"):
        s = DveOpSpec(
            name=name, opcode=opcode, uops=lower(spec, ver=ver), rd1_en=_has_src1(spec)
        )
        shas[ver] = s.sha(ver)
    op = dve_ops.DveOp(name, spec, subdim=subdim, uops_sha=shas)
    dve_ops.OPS.append(op)
    dve_ops._SUB_OPCODE_FOR_NAME[name] = opcode
    dve_ops.CUSTOM_DVE_SPECS[name] = spec
    return op


# ---- fused cumsum ops (z = (2t+1)*m; m = min(z,1), tm = relu((z-1)*0.5)) ----
def _z_parts(z):
    z = _f32(z)
    return np.minimum(z, 1.0), np.maximum((z - 1.0) * np.float32(0.5), 0.0)


def _ref_mscan(in0, in1, s0, s1, imm2):
    return np.cumsum(np.minimum(_f32(in0), 1.0), axis=-1, dtype=np.float32)


def _ref_gscan(in0, in1, s0, s1, imm2):
    m, tm = _z_parts(in1)
    return np.cumsum(_f32(in0) * _f32(in0) * m - tm, axis=-1, dtype=np.float32)


def _ref_hscan(in0, in1, s0, s1, imm2):
    m, tm = _z_parts(in1)
    return np.cumsum(_f32(in0) * m - tm, axis=-1, dtype=np.float32)


def _ref_kscan(in0, in1, s0, s1, imm2):
    m, tm = _z_parts(in1)
    return np.cumsum(_f32(in0) * m + tm, axis=-1, dtype=np.float32)


_m_of_z = minn(Src1, One)
_tm_of_z = relu((Src1 - One) * C0)  # s0 = 0.5

MSCAN = _register_op(
    "BCL_MSCAN", Spec(body=scan(AluOp.ADD, minn(Src0, One)), reference=_ref_mscan)
)
GSCAN = _register_op(
    "BCL_GSCAN",
    Spec(body=scan(AluOp.ADD, sq(Src0) * _m_of_z - _tm_of_z), reference=_ref_gscan),
)
def _ref_ascan(in0, in1, s0, s1, imm2):
    m, tm = _z_parts(in1)
    return np.cumsum(_f32(in0) * m, axis=-1, dtype=np.float32)


def _ref_bscan(in0, in1, s0, s1, imm2):
    m, tm = _z_parts(in0)
    return np.cumsum(tm, axis=-1, dtype=np.float32)


ASCAN = _register_op(
    "BCL_ASCAN", Spec(body=scan(AluOp.ADD, Src0 * _m_of_z), reference=_ref_ascan)
)
BSCAN = _register_op(
    "BCL_BSCAN",
    Spec(body=scan(AluOp.ADD, relu((Src0 - One) * C0)), reference=_ref_bscan),
)

# ---- r = 1/max(Src0-Src1, 1): NOT-seed + one Newton pass, constants
# minimax-fitted to the exact domain d in {1..5} (max rel err 0.13%).
RW_CONSTS = {"s0": -0.2231725650808697, "s1": 1.8969666701661758,
             "imm2": 1.1139888028328857}


def _ref_rwin(in0, in1, c0, c1, c2):
    d = np.maximum(_f32(in0) - _f32(in1), 1.0)
    not_d = (~d.view(np.int32)).view(np.float32)
    y0 = not_d * c0
    return y0 * (c1 - d * y0) * c2


_d = maxx(Src0 - Src1, One)
_y0 = Bin(AluOp.BITWISE_NOT, _d, _d) * C0
RWIN = _register_op(
    "BCL_RWIN", Spec(body=(_y0 * (C1 - _d * _y0)) * C2, reference=_ref_rwin)
)


def _split_sync_waits(nc, max_waits=1):
    """walrus TPB_CTRL codegen rejects >1 explicit sem wait on Drain-class
    instructions; move excess waits onto preceding same-engine no-ops."""
    for fn in nc.m.functions:
        for bb in fn.blocks:
            new_insts = []
            for ins in bb.instructions:
                si = getattr(ins, "sync_info", None)
                waits = list(si.on_wait) if si is not None else []
                if len(waits) > max_waits:
                    extra, keep = waits[:-max_waits], waits[-max_waits:]
                    for j in range(0, len(extra), max_waits):
                        new_insts.append(
                            mybir.InstNoOp(
                                name=f"{ins.name}_wsplit{j}",
                                engine=ins.engine,
                                ins=[],
                                outs=[],
                                sync_info=mybir.SyncInfo(
                                    on_wait=extra[j : j + max_waits], on_update=[]
                                ),
                            )
                        )
                    si.on_wait.clear()
                    si.on_wait.extend(keep)
                new_insts.append(ins)
            bb.instructions = new_insts


def _build_program():
    nc = bacc.Bacc(
        "TRN2",
        target_bir_lowering=False,
        debug=False,
        enable_asserts=False,
        num_devices=NCORES,
    )
    x0d = nc.dram_tensor("x0", [1, TLEN], F32, kind="ExternalInput")
    x1d = nc.dram_tensor("x1", [1, TLEN], F32, kind="ExternalInput")
    tg = nc.dram_tensor("targets", [1, TLEN], I32, kind="ExternalInput")
    mk = nc.dram_tensor("mask", [1, TLEN], I32, kind="ExternalInput")
    ones_in = nc.dram_tensor("ones_const", [P, 2], BF16, kind="ExternalInput")
    ssd_o = nc.dram_tensor("ssd", [2, LH], BF16, kind="ExternalOutput")
    mst_o = nc.dram_tensor("mst", [2, LH], BF16, kind="ExternalOutput")

    with tile.TileContext(nc) as tc:
        with (
            tc.tile_pool(name="const", bufs=1) as const,
            tc.tile_pool(name="wsp", bufs=1) as wsp,      # f32 scan tiles
            tc.tile_pool(name="mid", bufs=2) as mid,      # bf16 integrands
            tc.tile_pool(name="cmb", bufs=2) as cmb,      # bf16 combine tiles
            tc.tile_pool(name="ev", bufs=2) as ev,
            tc.tile_pool(name="ps", bufs=2, space="PSUM") as ps,
        ):
            x0t = const.tile([P, LH + 4], BF16)
            x1t = const.tile([P, LH + 4], BF16)
            tb = const.tile([P, LH + 4], BF16)
            mb = const.tile([P, LH + 4], BF16)
            ones = const.tile([P, 2], BF16)

            nc.sync.dma_start(out=ones[:, :], in_=ones_in[:, :])

            def i_ap(t, lo, hi):
                return bass.AP(tensor=t[:, :].tensor, offset=lo,
                               ap=[[LH, P], [1, hi - lo]])

            for i in range(len(TP) - 1):
                lo, hi = TP[i], TP[i + 1]
                nc.gpsimd.dma_start(out=tb[:, lo:hi], in_=i_ap(tg, lo, hi))
                nc.gpsimd.dma_start(out=mb[:, lo:hi], in_=i_ap(mk, lo, hi))
                nc.gpsimd.dma_start(out=x0t[:, lo:hi], in_=i_ap(x0d, lo, hi))
                nc.gpsimd.dma_start(out=x1t[:, lo:hi], in_=i_ap(x1d, lo, hi))

            for c0, CK in CHUNKS:
                CKH = CK + (W - 1)

                dsg = mid.tile([P, CKMAX + 4], BF16, tag="dsg")
                nc.vector.tensor_sub(
                    dsg[:, :CKH], x1t[:, c0 : c0 + CKH], x0t[:, c0 : c0 + CKH]
                )
                p_t = mid.tile([P, CKMAX + 4], BF16, tag="p")
                nc.scalar.activation(p_t[:, :CKH], dsg[:, :CKH], AF.Sigmoid)

                z2 = mid.tile([P, CKMAX + 4], BF16, tag="z2")
                nc.scalar.activation(z2[:, :CKH], tb[:, c0 : c0 + CKH],
                                     AF.Identity, bias=1.0, scale=2.0)
                z = mid.tile([P, CKMAX + 4], BF16, tag="z")
                nc.vector.tensor_mul(z[:, :CKH], z2[:, :CKH],
                                     mb[:, c0 : c0 + CKH])

                cts = {}
                for nm, op_, dt_, args in (
                    ("m", MSCAN, F32, dict(in0=z[:, :CKH])),
                    ("g", GSCAN, F32,
                     dict(in0=p_t[:, :CKH], in1=z[:, :CKH], s0=0.5)),
                    ("a", ASCAN, F32,
                     dict(in0=p_t[:, :CKH], in1=z[:, :CKH], s0=0.5)),
                    # tm cumsum is integer <= CKH: exact in fp16, so its
                    # window diff below runs as a packed-2-byte 2x op
                    ("b", BSCAN, F16, dict(in0=z[:, :CKH], s0=0.5)),
                ):
                    ct = wsp.tile([P, CKMAX + 5], dt_, tag=f"c_{nm}")
                    nc.vector.memset(ct[:, 0:1], 0.0)
                    nc.vector._custom_dve(op_, out=ct[:, 1 : CKH + 1], **args)
                    cts[nm] = ct

                r = cmb.tile([P, CKMAX], BF16, tag="r")
                nc.vector._custom_dve(
                    RWIN, out=r[:, :CK], in0=cts["m"][:, W : CK + W],
                    in1=cts["m"][:, 0:CK],
                    s0=RW_CONSTS["s0"], s1=RW_CONSTS["s1"], imm2=RW_CONSTS["imm2"],
                )
                Gm = cmb.tile([P, CKMAX], BF16, tag="Gm")
                nc.vector.tensor_sub(Gm[:, :CK], cts["g"][:, W : CK + W],
                                     cts["g"][:, 0:CK])
                Aw = cmb.tile([P, CKMAX], BF16, tag="Aw")
                nc.vector.tensor_sub(Aw[:, :CK], cts["a"][:, W : CK + W],
                                     cts["a"][:, 0:CK])
                Bw = cmb.tile([P, CKMAX], BF16, tag="Bw")
                nc.vector.tensor_sub(Bw[:, :CK], cts["b"][:, W : CK + W],
                                     cts["b"][:, 0:CK])
                A2 = cmb.tile([P, CKMAX], BF16, tag="A2")
                nc.scalar.square(A2[:, :CK], Aw[:, :CK])
                B2 = cmb.tile([P, CKMAX], BF16, tag="B2")
                nc.scalar.square(B2[:, :CK], Bw[:, :CK])

                W1 = cmb.tile([P, CKMAX], BF16, tag="W1")
                nc.vector.tensor_sub(W1[:, :CK], A2[:, :CK], B2[:, :CK])
                nc.vector.tensor_mul(W1[:, :CK], W1[:, :CK], r[:, :CK])
                nc.vector.tensor_sub(W1[:, :CK], Gm[:, :CK], W1[:, :CK])
                nc.vector.tensor_mul(W1[:, :CK], W1[:, :CK], r[:, :CK])
                d2 = cmb.tile([P, CKMAX], BF16, tag="d2")
                nc.scalar.square(d2[:, :CK], W1[:, :CK])

                NQ = max(CK // 1024, 1)
                QW = CK // NQ
                for q in range(NQ):
                    mst_ps = ps.tile([2, 1024], F32, tag="mstp")
                    ssd_ps = ps.tile([2, 1024], F32, tag="ssdp")
                    for h in range(QW // 512):
                        sl = slice(q * QW + h * 512, q * QW + (h + 1) * 512)
                        psl = slice(h * 512, (h + 1) * 512)
                        nc.tensor.matmul(mst_ps[:, psl], ones[:, :], Aw[:, sl],
                                         start=True, stop=True)
                        nc.tensor.matmul(ssd_ps[:, psl], ones[:, :], d2[:, sl],
                                         start=True, stop=True)
                    ssd_ev = ev.tile([2, 1024], BF16, tag="ssd_ev")
                    nc.scalar.copy(ssd_ev[:, :QW], ssd_ps[:, :QW])
                    mst_ev = ev.tile([2, 1024], BF16, tag="mst_ev")
                    nc.scalar.copy(mst_ev[:, :QW], mst_ps[:, :QW])
                    nc.sync.dma_start(
                        out=ssd_o[:, c0 + q * QW : c0 + (q + 1) * QW],
                        in_=ssd_ev[:, :QW])
                    nc.sync.dma_start(
                        out=mst_o[:, c0 + q * QW : c0 + (q + 1) * QW],
                        in_=mst_ev[:, :QW])

    nc.compile()
    nc.m = get_hw_module(nc.m)
    _split_sync_waits(nc)
    return nc


_NC_CACHE = {}


def _get_nc():
    if "nc" not in _NC_CACHE:
        _NC_CACHE["nc"] = _build_program()
    return _NC_CACHE["nc"]


def _ones_pattern():
    import ml_dtypes

    o = np.zeros((P, 2), np.float32)
    o[0::2, 0] = 1.0   # even partitions: h=0 rows
    o[1::2, 1] = 1.0   # odd partitions: h=1 rows
    return o.astype(ml_dtypes.bfloat16)


def run_on_device(predictions, targets, mask, **spmd_kwargs):
    nc = _get_nc()
    predictions = np.asarray(predictions, np.float32)
    targets = np.asarray(targets, np.int32)
    mask = np.asarray(mask, np.int32)
    ones = _ones_pattern()
    in_maps = []
    zp = np.zeros(W - 1, np.float32)
    zi = np.zeros(W - 1, np.int32)
    for i in range(NCORES):
        sl = slice(i * BL, (i + 1) * BL)
        in_maps.append(
            {
                "x0": np.concatenate(
                    [np.ascontiguousarray(predictions[sl, :, 0]).ravel(),
                     zp])[None, :],
                "x1": np.concatenate(
                    [np.ascontiguousarray(predictions[sl, :, 1]).ravel(),
                     zp])[None, :],
                "targets": np.concatenate([targets[sl].ravel(), zi])[None, :],
                "mask": np.concatenate([mask[sl].ravel(), zi])[None, :],
                "ones_const": ones,
            }
        )
    return run_bass_kernel_spmd(nc, in_maps, core_ids=list(range(NCORES)),
                                **spmd_kwargs)


def combine_host(results):
    ssd_tot = np.zeros(NW, np.float64)
    mst_tot = np.zeros(NW, np.float64)
    for out in results:
        ssd = np.asarray(out["ssd"], np.float32)
        mst = np.asarray(out["mst"], np.float32)
        ssd_tot += np.concatenate([ssd[0], ssd[1][: NW - LH]])
        mst_tot += np.concatenate([mst[0], mst[1][: NW - LH]])
    mse = ssd_tot / B
    valid = (mst_tot > 0).astype(np.float64)
    cnt = max(valid.sum(), 1.0)
    loss = (mse * valid).sum() / cnt
    return np.asarray(loss, dtype=np.float32)


def kernel(predictions, targets, mask):
    res = run_on_device(predictions, targets, mask)
    return combine_host(res.results)


if __name__ == "__main__":
    rng = np.random.default_rng(0)
    p = rng.standard_normal((B, L, C), dtype=np.float32)
    t = rng.integers(0, 2, (B, L)).astype(np.int32)
    m = rng.integers(0, 2, (B, L)).astype(np.int32)
    print(kernel(p, t, m))
